# revision 1
# baseline (speedup 1.0000x reference)
"""Trainium2 Bass kernel for nn_DglAggregator (GNN message passing).

Strategy (8 NeuronCores, SPMD, one uniform program, per-core data):
- Targets are partitioned across cores balanced by stage-1 edge count; each
  core owns its targets' items and ALL stage-1 edges pointing at those items,
  so no cross-core communication is needed.
- Stage 1 (item->item segment softmax + weighted sum): edges sorted by dst
  item; per 1024-edge window the per-edge weights exp(score) are folded into
  a [128 edges x 128 slots] selection matrix and accumulated into per-window
  PSUM via TensorE matmuls (unnormalized sums + denominators); normalization
  is a per-slot row scale at readout. Softmax max-subtraction is skipped:
  scores are O(1) here (|score| < ~6) so exp is exact-safe in f32.
- The x_src rows are fetched with the int16 dma_gather ucode. The global
  table (200k rows) exceeds int16, so gathers are issued per 32768-row chunk
  (edges of each batch grouped by chunk, quota-padded), staged through a DRAM
  scratch, and re-gathered (int16 positions) into dst-sorted order.
- The x_dst rows come from a per-core local table (item-slot order, <=32k
  rows -> single int16 gather), pre-scaled by pi on device.
- Stage 2 (item->target): mean/deg, f = [h_t, mean] @ r_w, e2 = tanh([ft,
  h_p] @ q_w), w = <e2, f[dst]>, out = sum w*ft — all via the same masked
  matmul pattern over 128-target windows; per-edge f rows via int16 gather
  from an on-device f table.
- Host-side work is limited to graph restructuring: integer index math and
  row permutations of input tables (sharding). All floating-point arithmetic
  on the data path runs on the NeuronCores.

kernel(**inputs) accepts the FULL unsharded inputs and returns the FULL
[N_TGT, 128] output.
"""
import numpy as np

P = 128          # partitions / tile edge
D = 128          # feature dim
NCORES = 8
CHUNK = 32768    # int16-addressable table chunk
WE1 = 1024       # stage-1 window edge capacity (8 tiles)
WS1 = 128        # stage-1 window slot capacity
WB = 8           # stage-1 windows per batch
TI2 = 50         # stage-2 tiles per window (6400 item slots)
WS2 = 128        # stage-2 window target capacity
GH = 4096        # gather granularity for g2/xd (half batch)
BUFS = {"xs": 2, "gat": 2, "wk": 3, "sm": 4, "pp": 2, "ip1": 2, "p0": 3,
        "bg": 2, "wk2": 3, "ppB": 2}
PHASES = 2       # debug: 0 = P0 only, 1 = +stage-1, 2 = full
_LAST_NC = None
P1SUB = "full"   # debug: "gather" | "dve" | "full"
ABL = set()      # timing ablations: g1 scr g2 xd scores dvemask mm


def _wrap_idx16(idx: np.ndarray, cap: int) -> np.ndarray:
    """[n<=cap] -> [128, cap/16] int16 (j at [j%16, j//16], replicated x8).
    Pad with 0 (row 0 is always a valid gather target)."""
    a = np.zeros(cap, np.int64)
    a[: idx.shape[0]] = idx
    assert cap % 16 == 0
    assert a.min() >= 0 and a.max() < 32768, (a.min(), a.max())
    blk = a.reshape(cap // 16, 16).T.astype(np.int16)
    return np.tile(blk, (8, 1))


def _interleave_f32(vals: np.ndarray, cap: int, fill: float) -> np.ndarray:
    """[n] -> [128, cap/128] f32 with value of rank r at [r%128, r//128]."""
    a = np.full(cap, fill, np.float32)
    a[: vals.shape[0]] = vals
    return a.reshape(cap // P, P).T.copy()


def _pack_runs(run_sizes, max_runs, max_total):
    """Greedy pack consecutive runs into groups: each group holds whole runs,
    <= max_runs runs and <= max_total total size. Oversized single runs are
    rejected (assert). Returns list of (start_run, n_runs)."""
    groups = []
    i, n = 0, len(run_sizes)
    while i < n:
        tot, j = 0, i
        while j < n and j - i < max_runs and tot + run_sizes[j] <= max_total:
            tot += run_sizes[j]
            j += 1
        assert j > i, f"run {i} of size {run_sizes[i]} exceeds {max_total}"
        groups.append((i, j - i))
        i = j
    return groups


def preprocess(h_v, h_p, h_t, int_src, int_dst, agg_dst):
    """All graph restructuring. Returns shared dims + per-core arrays."""
    NITEM = h_v.shape[0]
    NTGT = h_t.shape[0]
    int_src = int_src.astype(np.int64)
    int_dst = int_dst.astype(np.int64)
    item_tgt = agg_dst.astype(np.int64)          # item i -> target (agg_src=arange)
    n_chunks = (NITEM + CHUNK - 1) // CHUNK

    # ---- target -> core, balanced by stage-1 edge load ----
    deg_int = np.bincount(int_dst, minlength=NITEM)
    t_edges = np.bincount(item_tgt, weights=deg_int.astype(np.float64),
                          minlength=NTGT)
    t_items = np.bincount(item_tgt, minlength=NTGT)
    tgt_core = np.zeros(NTGT, np.int64)
    load = np.zeros(NCORES)
    for t in np.argsort(-t_edges, kind="stable"):
        c = int(np.argmin(load))
        tgt_core[t] = c
        load[c] += t_edges[t] + 0.5 * t_items[t]

    item_core = tgt_core[item_tgt]

    cores = []
    for c in range(NCORES):
        tlist = np.where(tgt_core == c)[0]
        items = np.where(item_core == c)[0]
        # items ordered by (target, item id)
        items = items[np.lexsort((items, item_tgt[items]))]
        cores.append({"targets": tlist, "items": items})

    # ---- stage-2 windows (whole targets, <=WS2 targets, <=TI2*128 islots) ----
    for c in range(NCORES):
        st = cores[c]
        tl = st["targets"]
        sizes = t_items[tl]
        groups = _pack_runs(sizes, WS2, TI2 * P)
        st["w2groups"] = groups
    W2 = max(len(st["w2groups"]) for st in cores)
    NI = W2 * TI2 * P

    for c in range(NCORES):
        st = cores[c]
        tl, items = st["targets"], st["items"]
        it_item = np.full(NI, -1, np.int64)       # islot -> global item
        it_tgtloc = np.full(NI, -1.0, np.float32)  # islot -> window-local tgt
        it_tslot = np.zeros(NI, np.int64)          # islot -> global tgt slot
        twin = np.full((W2, WS2), -1, np.int64)    # window -> global targets
        ipos = 0  # position within items array
        for w2, (t0, ntgt) in enumerate(st["w2groups"]):
            base = w2 * TI2 * P
            off = 0
            for k in range(ntgt):
                t = tl[t0 + k]
                cnt = int(t_items[t])
                sl = slice(base + off, base + off + cnt)
                it_item[sl] = items[ipos : ipos + cnt]
                it_tgtloc[sl] = k
                it_tslot[sl] = w2 * WS2 + k
                twin[w2, k] = t
                ipos += cnt
                off += cnt
        assert ipos == len(items)
        st["it_item"] = it_item
        st["it_tgtloc"] = it_tgtloc
        st["it_tslot"] = it_tslot
        st["twin"] = twin
        islot_of = np.full(NITEM, -1, np.int64)
        real = it_item >= 0
        islot_of[it_item[real]] = np.where(real)[0]
        st["islot_of"] = islot_of

    # ---- stage-1 edges, windows ----
    for c in range(NCORES):
        st = cores[c]
        emask = item_core[int_dst] == c
        es = int_src[emask]
        ed = st["islot_of"][int_dst[emask]]
        o = np.argsort(ed, kind="stable")
        es, ed = es[o], ed[o]
        # windows over whole dst-slot runs
        uslots, ustart, ucnt = np.unique(ed, return_index=True, return_counts=True)
        groups = _pack_runs(ucnt, WS1, WE1)
        st["e_src"] = es
        st["e_dst"] = ed
        st["w1groups"] = groups
        st["uslots"] = uslots
        st["ustart"] = ustart
        st["ucnt"] = ucnt
    W1 = max(len(st["w1groups"]) for st in cores)
    W1 = ((W1 + WB - 1) // WB) * WB
    B1 = W1 // WB
    assert W1 * WS1 + P <= 32768, f"FT table too big for int16: W1={W1}"

    # per-window edge/seg arrays (original w1groups order)
    for c in range(NCORES):
        st = cores[c]
        es, ed = st["e_src"], st["e_dst"]
        uslots, ustart, ucnt = st["uslots"], st["ustart"], st["ucnt"]
        # pad-edge sources spread round-robin across chunks so quota
        # padding doesn't concentrate in chunk 0 (pad rows are gathered but
        # never consumed: their positions are masked via seg=-1)
        nspread = max(1, n_chunks - 1)
        wsrc = (np.arange(WE1, dtype=np.int64)[None, :] % nspread) * CHUNK             + np.zeros((W1, 1), np.int64)
        wdst = np.zeros((W1, WE1), np.int64)       # dst islot (pad 0)
        wseg = np.full((W1, WE1), -1.0, np.float32)  # window-local slot (pad -1)
        wcnt = np.zeros(W1, np.int64)
        for w, (r0, nr) in enumerate(st["w1groups"]):
            e0 = ustart[r0]
            ne = int(ucnt[r0 : r0 + nr].sum())
            wsrc[w, :ne] = es[e0 : e0 + ne]
            wdst[w, :ne] = ed[e0 : e0 + ne]
            lab = np.repeat(np.arange(nr), ucnt[r0 : r0 + nr])
            wseg[w, :ne] = lab
            wcnt[w] = ne
        st["wsrc"], st["wdst"], st["wseg"], st["wcnt"] = wsrc, wdst, wseg, wcnt

    # ---- batches: assign windows to batches balancing chunk quotas, then
    # renumber windows by (batch, rank-in-batch) so FT readout offsets are
    # uniform code across cores ----
    for c in range(NCORES):
        st = cores[c]
        wsrc, wcnt = st["wsrc"], st["wcnt"]
        ck = wsrc // CHUNK
        cnts = np.zeros((W1, n_chunks), np.int64)
        for k in range(n_chunks):
            cnts[:, k] = (ck == k).sum(1)
        # pads counted in chunk 0 — treat as real work for quota purposes
        order = np.argsort(-cnts.max(1), kind="stable")
        bload = np.zeros((B1, n_chunks), np.int64)
        bfill = np.zeros(B1, np.int64)
        wbatch = np.zeros(W1, np.int64)
        for w in order:
            cand = np.where(bfill < WB)[0]
            j = cand[np.argmin((bload[cand] + cnts[w]).max(1))]
            wbatch[w] = j
            bload[j] += cnts[w]
            bfill[j] += 1
        # swap refinement: move window pairs between batches when it lowers
        # the global max per-chunk load (which sets the shared quotas)
        for _ in range(4):
            improved = False
            worst = int(np.argmax(bload.max(1)))
            ws_w = np.where(wbatch == worst)[0]
            for ob in np.argsort(bload.max(1))[: B1 // 2]:
                if ob == worst:
                    continue
                ws_o = np.where(wbatch == ob)[0]
                cur = max(bload[worst].max(), bload[ob].max())
                done = False
                for a in ws_w:
                    for b in ws_o:
                        nw = bload[worst] - cnts[a] + cnts[b]
                        no = bload[ob] - cnts[b] + cnts[a]
                        if max(nw.max(), no.max()) < cur:
                            bload[worst], bload[ob] = nw, no
                            wbatch[a], wbatch[b] = ob, worst
                            improved = done = True
                            break
                    if done:
                        break
                if done:
                    break
            if not improved:
                break
        st["bload"] = bload
        # new order: stable sort by batch; window new index = position
        neww = np.argsort(wbatch, kind="stable")   # new_idx -> old_idx
        st["wsrc"] = st["wsrc"][neww]
        st["wdst"] = st["wdst"][neww]
        st["wseg"] = st["wseg"][neww]
        st["wcnt"] = st["wcnt"][neww]
        # ft slots follow the NEW window numbering
        ft_slot = np.full(NI, W1 * WS1, np.int64)  # default: zero page
        old2new = np.argsort(neww, kind="stable")
        uslots = st["uslots"]
        for wold, (r0, nr) in enumerate(st["w1groups"]):
            wnew = old2new[wold]
            ft_slot[uslots[r0 : r0 + nr]] = wnew * WS1 + np.arange(nr)
        st["ft_slot"] = ft_slot
    # shared quotas (multiple of 128, >=128)
    Q = np.zeros(n_chunks, np.int64)
    for c in range(NCORES):
        Q = np.maximum(Q, cores[c]["bload"].max(0))
    # 256-multiples keep every idx-tile slice offset 32B-aligned for the
    # gather ucode
    Q = np.maximum(((Q + 255) // 256) * 256, 256)
    SC = int(Q.sum())
    Qoff = np.concatenate([[0], np.cumsum(Q)])

    # ---- per-batch gather arrays (batch b = windows [b*WB, (b+1)*WB)) ----
    for c in range(NCORES):
        st = cores[c]
        g1 = np.zeros((B1, P, SC // 16), np.int16)
        g2 = np.zeros((B1, P, (WB * WE1) // 16), np.int16)
        xd = np.zeros((B1, P, (WB * WE1) // 16), np.int16)
        seg = np.full((B1, P, (WB * WE1) // P), -1.0, np.float32)
        for b in range(B1):
            wins = np.arange(b * WB, (b + 1) * WB)
            src = st["wsrc"][wins].reshape(-1)        # [WB*WE1] rank order
            dst = st["wdst"][wins].reshape(-1)
            sg = st["wseg"][wins].reshape(-1)
            ck = src // CHUNK
            pos = np.zeros(WB * WE1, np.int64)
            g1i = np.zeros(SC, np.int64)
            for k in range(n_chunks):
                sel = np.where(ck == k)[0]
                assert len(sel) <= Q[k], (c, b, k, len(sel), Q[k])
                pos[sel] = Qoff[k] + np.arange(len(sel))
                g1i[Qoff[k] : Qoff[k] + len(sel)] = src[sel] - k * CHUNK
            g1[b] = _wrap_idx16(g1i, SC)
            g2[b] = _wrap_idx16(pos, WB * WE1)
            xd[b] = _wrap_idx16(dst, WB * WE1)
            seg[b] = _interleave_f32(sg, WB * WE1, -1.0)
        st["g1"], st["g2"], st["xd"], st["seg"] = g1, g2, xd, seg

    # ---- stage-2 gather/meta arrays + tables ----
    for c in range(NCORES):
        st = cores[c]
        it_item = st["it_item"]
        real = it_item >= 0
        st["ftg"] = _wrap_idx16(st["ft_slot"], NI)
        st["fexp"] = _wrap_idx16(st["it_tslot"], NI)
        tl = np.zeros((W2, P, TI2), np.float32)
        for w2 in range(W2):
            tl[w2] = _interleave_f32(
                st["it_tgtloc"][w2 * TI2 * P : (w2 + 1) * TI2 * P], TI2 * P, -1.0
            )
        st["tgtloc"] = tl
        hv_local = np.zeros((NI, D), np.float32)
        hv_local[real] = h_v[it_item[real]]
        st["hv_local"] = hv_local
        hpT = np.zeros((D, NI), np.float32)
        hpT[:, real] = h_p[it_item[real]].T
        st["hpT"] = hpT
        htT = np.zeros((D, W2 * WS2), np.float32)
        tw = st["twin"].reshape(-1)
        htT[:, tw >= 0] = h_t[tw[tw >= 0]].T
        st["htT"] = htT

    dims = {
        "NI": NI, "W1": W1, "B1": B1, "W2": W2, "SC": SC,
        "Q": Q.tolist(), "Qoff": Qoff.tolist(), "n_chunks": n_chunks,
        "NITEM": NITEM, "NTGT": NTGT,
    }
    return dims, cores


# ======================= device program =======================

def build_program(dims):
    import concourse.bacc as bacc
    import concourse.mybir as mybir
    import concourse.tile as tile

    f32 = mybir.dt.float32
    i16 = mybir.dt.int16
    Alu = mybir.AluOpType
    Act = mybir.ActivationFunctionType
    Ax = mybir.AxisListType

    NI, W1, B1, W2, SC = (dims[k] for k in ("NI", "W1", "B1", "W2", "SC"))
    Q, Qoff, n_chunks = dims["Q"], dims["Qoff"], dims["n_chunks"]
    NITEM = dims["NITEM"]
    RB = WB * WE1          # ranks per batch (8192)

    nc = bacc.Bacc("TRN2", target_bir_lowering=False, debug=False,
                   num_devices=NCORES)
    # inputs
    hv = nc.dram_tensor("hv", [NITEM, D], f32, kind="ExternalInput")
    hvl = nc.dram_tensor("hvl", [NI, D], f32, kind="ExternalInput")
    hpT = nc.dram_tensor("hpT", [D, NI], f32, kind="ExternalInput")
    htT = nc.dram_tensor("htT", [D, W2 * WS2], f32, kind="ExternalInput")
    qw = nc.dram_tensor("qw", [2 * D, D], f32, kind="ExternalInput")
    rw = nc.dram_tensor("rw", [2 * D, D], f32, kind="ExternalInput")
    pirep = nc.dram_tensor("pirep", [P, 4 * D], f32, kind="ExternalInput")
    iota = nc.dram_tensor("iota", [P, P], f32, kind="ExternalInput")
    ident = nc.dram_tensor("ident", [P, P], f32, kind="ExternalInput")
    g1d = nc.dram_tensor("g1d", [B1, P, SC // 16], i16, kind="ExternalInput")
    g2d = nc.dram_tensor("g2d", [B1, P, RB // 16], i16, kind="ExternalInput")
    xdd = nc.dram_tensor("xdd", [B1, P, RB // 16], i16, kind="ExternalInput")
    segd = nc.dram_tensor("segd", [B1, P, RB // P], f32, kind="ExternalInput")
    ftgd = nc.dram_tensor("ftgd", [P, NI // 16], i16, kind="ExternalInput")
    fexpd = nc.dram_tensor("fexpd", [P, NI // 16], i16, kind="ExternalInput")
    tgtlocd = nc.dram_tensor("tgtlocd", [W2, P, TI2], f32, kind="ExternalInput")
    # output
    outd = nc.dram_tensor("out", [W2 * WS2, D], f32, kind="ExternalOutput")
    # internal scratch
    xsd_dram = nc.dram_tensor("xs", [SC, D], f32, kind="Internal")
    hvpi = nc.dram_tensor("hvpi", [NI, D], f32, kind="Internal")
    ftd = nc.dram_tensor("ft", [W1 * WS1 + P, D], f32, kind="Internal")
    fd = nc.dram_tensor("fd", [W2 * WS2, D], f32, kind="Internal")

    with tile.TileContext(nc) as tc:
        with (
            tc.tile_pool(name="consts", bufs=1) as cp,
            tc.tile_pool(name="weights", bufs=1) as wp,
        ):
            iota_t = cp.tile([P, P], f32)
            nc.sync.dma_start(out=iota_t[:], in_=iota[:])
            ident_t = cp.tile([P, P], f32)
            nc.sync.dma_start(out=ident_t[:], in_=ident[:])
            pirep_t = cp.tile([P, 4 * D], f32)
            nc.sync.dma_start(out=pirep_t[:], in_=pirep[:])
            ones_t = cp.tile([P, 1], f32)
            nc.vector.memset(ones_t[:], 1.0)
            qwa = wp.tile([D, D], f32)
            nc.sync.dma_start(out=qwa[:], in_=qw[0:D, :])
            qwb = wp.tile([D, D], f32)
            nc.sync.dma_start(out=qwb[:], in_=qw[D : 2 * D, :])
            rwa = wp.tile([D, D], f32)
            nc.sync.dma_start(out=rwa[:], in_=rw[0:D, :])
            rwb = wp.tile([D, D], f32)
            nc.sync.dma_start(out=rwb[:], in_=rw[D : 2 * D, :])

            # ---- P0: hvpi = hvl * pi (row blocks), FT zero page ----
            with tc.tile_pool(name="p0", bufs=BUFS["p0"]) as p0:
                zt = p0.tile([P, D], f32, tag="zt")
                nc.vector.memset(zt[:], 0.0)
                nc.sync.dma_start(out=ftd[W1 * WS1 :, :], in_=zt[:])
                nblk = NI // P
                for i0 in range(0, nblk, 4):
                    nb = min(4, nblk - i0)
                    t = p0.tile([P, nb, D], f32, tag="hv")
                    nc.sync.dma_start(
                        out=t[:],
                        in_=hvl[i0 * P : (i0 + nb) * P, :].rearrange(
                            "(c p) d -> p c d", p=P))
                    u = p0.tile([P, nb, D], f32, tag="u")
                    nc.vector.tensor_tensor(
                        out=u[:], in0=t[:],
                        in1=pirep_t[:, : nb * D].rearrange("p (c d) -> p c d", c=nb),
                        op=Alu.mult)
                    nc.sync.dma_start(
                        out=hvpi[i0 * P : (i0 + nb) * P, :].rearrange(
                            "(c p) d -> p c d", p=P), in_=u[:])

            # ---- P1: stage-1 batches ----
            with (
                tc.tile_pool(name="idx1", bufs=BUFS["ip1"]) as ip1,
                tc.tile_pool(name="xs", bufs=BUFS["xs"]) as xsp,
                tc.tile_pool(name="gat", bufs=BUFS["gat"]) as gp,
                tc.tile_pool(name="work1", bufs=BUFS["wk"]) as wk,
                tc.tile_pool(name="small1", bufs=BUFS["sm"]) as sm,
                tc.tile_pool(name="psum1", bufs=BUFS["pp"], space="PSUM") as pp,
            ):
                for b in range(B1 if PHASES >= 1 else 0):
                    g1t = ip1.tile([P, SC // 16], i16, tag="g1")
                    nc.sync.dma_start(out=g1t[:], in_=g1d[b])
                    g2t = ip1.tile([P, RB // 16], i16, tag="g2")
                    nc.sync.dma_start(out=g2t[:], in_=g2d[b])
                    xdt = ip1.tile([P, RB // 16], i16, tag="xd")
                    nc.sync.dma_start(out=xdt[:], in_=xdd[b])
                    segt = ip1.tile([P, RB // P], f32, tag="seg")
                    nc.sync.dma_start(out=segt[:], in_=segd[b])

                    xs_sb = xsp.tile([P, SC // P, D], f32, tag="xs")
                    if "g1" in ABL:
                        nc.vector.memset(xs_sb[:, 0, 0:2], 0.0)
                    for k in range(n_chunks if "g1" not in ABL else 0):
                        lo = k * CHUNK
                        hi = min(NITEM, (k + 1) * CHUNK)
                        nc.gpsimd.dma_gather(
                            out_ap=xs_sb[:, Qoff[k] // P : Qoff[k + 1] // P, :],
                            in_ap=hv[lo:hi],
                            idxs_ap=g1t[:, Qoff[k] // 16 : Qoff[k + 1] // 16],
                            num_idxs=Q[k], num_idxs_reg=Q[k], elem_size=D, single_packet=False,
                        )
                    if "scr" not in ABL:
                        nc.sync.dma_start(
                            out=xsd_dram[:].rearrange("(c p) d -> p c d", p=P),
                            in_=xs_sb[:],
                        )
                    for h in range(2):
                        xsg = gp.tile([P, GH // P, D], f32, tag="xsg")
                        if "g2" in ABL:
                            nc.vector.memset(xsg[:, 0, 0:2], 0.0)
                        if "g2" not in ABL:
                            nc.gpsimd.dma_gather(
                                out_ap=xsg[:], in_ap=xsd_dram[:],
                                idxs_ap=g2t[:, h * GH // 16 : (h + 1) * GH // 16],
                                num_idxs=GH, num_idxs_reg=GH, elem_size=D, single_packet=False,
                            )
                        xdg = gp.tile([P, GH // P, D], f32, tag="xdg")
                        if "xd" in ABL:
                            nc.vector.memset(xdg[:, 0, 0:2], 0.0)
                        if "xd" not in ABL:
                            nc.gpsimd.dma_gather(
                                out_ap=xdg[:], in_ap=hvpi[:],
                                idxs_ap=xdt[:, h * GH // 16 : (h + 1) * GH // 16],
                                num_idxs=GH, num_idxs_reg=GH, elem_size=D, single_packet=False,
                            )
                        if "scores" in ABL:
                            pp_dummy_rd = sm.tile([P, 2], f32, tag="dr1")
                            pp_dummy_rd2 = sm.tile([P, 2], f32, tag="dr2")
                        for wl in range(GH // WE1 if P1SUB != "gather" else 0):
                            w = h * (GH // WE1) + wl
                            wg = b * WB + w                # global window
                            ftp = pp.tile([P, D], f32, space="PSUM", tag="ft")
                            denp = pp.tile([P, 1], f32, space="PSUM", tag="den")
                            if "scores" in ABL:
                                nc.vector.memset(ftp[:, 0:2], 0.0)
                                nc.vector.memset(denp[:], 0.0)
                                nc.vector.tensor_copy(out=pp_dummy_rd[:], in_=xsg[:, wl * 8, 0:2])
                                nc.vector.tensor_copy(out=pp_dummy_rd2[:], in_=xdg[:, wl * 8, 0:2])
                            for g in range(2 if "scores" not in ABL else 0):
                                t0 = wl * 8 + g * 4        # tile in half
                                scr3 = wk.tile([P, 4, D], f32, tag="scr3")
                                nc.vector.tensor_tensor(
                                    out=scr3[:], in0=xsg[:, t0 : t0 + 4, :],
                                    in1=xdg[:, t0 : t0 + 4, :], op=Alu.mult)
                                sc3 = sm.tile([P, 4], f32, tag="sc3")
                                nc.vector.tensor_reduce(
                                    out=sc3[:], in_=scr3[:], axis=Ax.X,
                                    op=Alu.add)
                                ex4 = sm.tile([P, 4], f32, tag="ex4")
                                nc.scalar.activation(out=ex4[:], in_=sc3[:],
                                                     func=Act.Exp)
                                for j in range(4 if P1SUB == "full" else 0):
                                    i = g * 4 + j          # tile in window
                                    col = w * 8 + i        # seg column in batch
                                    mask = wk.tile([P, P], f32, tag="mask")
                                    nc.vector.tensor_tensor(
                                        out=mask[:],
                                        in0=segt[:, col : col + 1].to_broadcast([P, P]),
                                        in1=iota_t[:], op=Alu.is_equal)
                                    maskex = wk.tile([P, P], f32, tag="maskex")
                                    nc.vector.tensor_scalar_mul(
                                        maskex[:], mask[:], ex4[:, j : j + 1])
                                    nc.tensor.matmul(
                                        out=ftp[:], lhsT=maskex[:],
                                        rhs=xsg[:, t0 + j, :],
                                        start=(i == 0), stop=(i == 7))
                                    nc.tensor.matmul(
                                        out=denp[:], lhsT=maskex[:],
                                        rhs=ones_t[:],
                                        start=(i == 0), stop=(i == 7))
                            if P1SUB != "full":
                                continue
                            denc = sm.tile([P, 1], f32, tag="denc")
                            nc.vector.tensor_scalar_max(denc[:], denp[:], 1e-30)
                            rec = sm.tile([P, 1], f32, tag="rec")
                            nc.vector.reciprocal(rec[:], denc[:])
                            ftsb = wk.tile([P, D], f32, tag="ftsb")
                            nc.vector.tensor_scalar_mul(ftsb[:], ftp[:], rec[:])
                            nc.sync.dma_start(
                                out=ftd[wg * WS1 : wg * WS1 + P, :], in_=ftsb[:])

            # ---- P2: stage-2 windows ----
            with (
                tc.tile_pool(name="idx2", bufs=1) as ip2,
                tc.tile_pool(name="big2", bufs=BUFS["bg"]) as bg,
                tc.tile_pool(name="work2", bufs=BUFS["wk2"]) as wk2,
                tc.tile_pool(name="small2", bufs=4) as sm2,
                tc.tile_pool(name="psumA", bufs=1, space="PSUM") as ppA,
                tc.tile_pool(name="psumB", bufs=BUFS["ppB"], space="PSUM") as ppB,
            ):
                ftgt = ip2.tile([P, NI // 16], i16, tag="ftg")
                nc.sync.dma_start(out=ftgt[:], in_=ftgd[:])
                fext = ip2.tile([P, NI // 16], i16, tag="fex")
                nc.sync.dma_start(out=fext[:], in_=fexpd[:])
                NW = TI2 * P                       # islots per window
                for w2 in range(W2 if PHASES >= 2 else 0):
                    hpt = bg.tile([P, NW], f32, tag="hpt")
                    nc.sync.dma_start(out=hpt[:],
                                      in_=hpT[:, w2 * NW : (w2 + 1) * NW])
                    tlt = ip2.tile([P, TI2], f32, tag="tlt")
                    nc.sync.dma_start(out=tlt[:], in_=tgtlocd[w2])
                    ftg = bg.tile([P, TI2, D], f32, tag="ftgw")
                    for o0, n in ((0, 4096), (4096, NW - 4096)):
                        o = w2 * NW + o0
                        nc.gpsimd.dma_gather(
                            out_ap=ftg[:, o0 // P : (o0 + n) // P, :],
                            in_ap=ftd[:],
                            idxs_ap=ftgt[:, o // 16 : (o + n) // 16],
                            num_idxs=n, num_idxs_reg=n, elem_size=D, single_packet=False)
                    # sweep A: mean + deg
                    meanp = ppA.tile([P, D], f32, space="PSUM", tag="mean")
                    degp = ppA.tile([P, 1], f32, space="PSUM", tag="deg")
                    for i in range(TI2):
                        mask = wk2.tile([P, P], f32, tag=f"maskA")
                        nc.vector.tensor_tensor(
                            out=mask[:],
                            in0=tlt[:, i : i + 1].to_broadcast([P, P]),
                            in1=iota_t[:], op=Alu.is_equal)
                        nc.tensor.matmul(out=meanp[:], lhsT=mask[:],
                                         rhs=ftg[:, i, :],
                                         start=(i == 0), stop=(i == TI2 - 1))
                        nc.tensor.matmul(out=degp[:], lhsT=mask[:],
                                         rhs=ones_t[:],
                                         start=(i == 0), stop=(i == TI2 - 1))
                    degc = sm2.tile([P, 1], f32, tag="degc")
                    nc.vector.tensor_scalar_max(degc[:], degp[:], 1.0)
                    rec2 = sm2.tile([P, 1], f32, tag="rec2")
                    nc.vector.reciprocal(rec2[:], degc[:])
                    mean_sb = wk2.tile([P, D], f32, tag="mean_sb")
                    nc.vector.tensor_scalar_mul(mean_sb[:], meanp[:], rec2[:])
                    # meanT + f
                    trp = ppB.tile([P, P], f32, space="PSUM", tag="trx")
                    nc.tensor.transpose(out=trp[:], in_=mean_sb[:],
                                        identity=ident_t[:])
                    meanT = wk2.tile([P, P], f32, tag="meanT")
                    nc.scalar.copy(out=meanT[:], in_=trp[:])
                    htt = wk2.tile([P, P], f32, tag="htt")
                    nc.sync.dma_start(out=htt[:],
                                      in_=htT[:, w2 * WS2 : (w2 + 1) * WS2])
                    fp = ppA.tile([P, D], f32, space="PSUM", tag="fp")
                    nc.tensor.matmul(out=fp[:], lhsT=htt[:], rhs=rwa[:],
                                     start=True, stop=False)
                    nc.tensor.matmul(out=fp[:], lhsT=meanT[:], rhs=rwb[:],
                                     start=False, stop=True)
                    f_sb = wk2.tile([P, D], f32, tag="f_sb")
                    nc.scalar.copy(out=f_sb[:], in_=fp[:])
                    nc.sync.dma_start(out=fd[w2 * WS2 : (w2 + 1) * WS2, :],
                                      in_=f_sb[:])
                    # sweep B
                    fex = bg.tile([P, TI2, D], f32, tag="fexw")
                    for o0, n in ((0, 4096), (4096, NW - 4096)):
                        o = w2 * NW + o0
                        nc.gpsimd.dma_gather(
                            out_ap=fex[:, o0 // P : (o0 + n) // P, :],
                            in_ap=fd[:],
                            idxs_ap=fext[:, o // 16 : (o + n) // 16],
                            num_idxs=n, num_idxs_reg=n, elem_size=D, single_packet=False)
                    outp = ppA.tile([P, D], f32, space="PSUM", tag="outp")
                    for i in range(TI2):
                        trp2 = ppB.tile([P, P], f32, space="PSUM", tag="trx")
                        nc.tensor.transpose(out=trp2[:], in_=ftg[:, i, :],
                                            identity=ident_t[:])
                        ftgT = wk2.tile([P, P], f32, tag="ftgT")
                        nc.scalar.copy(out=ftgT[:], in_=trp2[:])
                        e2p = ppB.tile([P, D], f32, space="PSUM", tag="e2p")
                        nc.tensor.matmul(out=e2p[:], lhsT=ftgT[:], rhs=qwa[:],
                                         start=True, stop=False)
                        nc.tensor.matmul(out=e2p[:],
                                         lhsT=hpt[:, i * P : (i + 1) * P],
                                         rhs=qwb[:], start=False, stop=True)
                        e2sb = wk2.tile([P, D], f32, tag="e2sb")
                        nc.scalar.activation(out=e2sb[:], in_=e2p[:],
                                             func=Act.Tanh)
                        scr = wk2.tile([P, D], f32, tag="scrB")
                        nc.vector.tensor_tensor(out=scr[:], in0=e2sb[:],
                                                in1=fex[:, i, :], op=Alu.mult)
                        wc = sm2.tile([P, 1], f32, tag="wc")
                        nc.vector.tensor_reduce(out=wc[:], in_=scr[:],
                                                axis=Ax.X, op=Alu.add)
                        maskB = wk2.tile([P, P], f32, tag="maskB")
                        nc.vector.tensor_tensor(
                            out=maskB[:],
                            in0=tlt[:, i : i + 1].to_broadcast([P, P]),
                            in1=iota_t[:], op=Alu.is_equal)
                        maskw = wk2.tile([P, P], f32, tag="maskw")
                        nc.vector.tensor_scalar_mul(maskw[:], maskB[:], wc[:])
                        nc.tensor.matmul(out=outp[:], lhsT=maskw[:],
                                         rhs=ftg[:, i, :],
                                         start=(i == 0), stop=(i == TI2 - 1))
                    out_sb = wk2.tile([P, D], f32, tag="out_sb")
                    nc.scalar.copy(out=out_sb[:], in_=outp[:])
                    nc.sync.dma_start(out=outd[w2 * WS2 : (w2 + 1) * WS2, :],
                                      in_=out_sb[:])
    nc.compile()
    return nc


def make_in_maps(dims, cores, h_v, pi_w, q_w, r_w):
    iota = np.tile(np.arange(P, dtype=np.float32), (P, 1))
    ident = np.eye(P, dtype=np.float32)
    pirep = np.tile(pi_w.reshape(1, D).astype(np.float32), (P, 4))
    in_maps = []
    for c in range(NCORES):
        st = cores[c]
        in_maps.append({
            "hv": np.ascontiguousarray(h_v, np.float32),
            "hvl": st["hv_local"],
            "hpT": st["hpT"],
            "htT": st["htT"],
            "qw": np.ascontiguousarray(q_w, np.float32),
            "rw": np.ascontiguousarray(r_w, np.float32),
            "pirep": pirep, "iota": iota, "ident": ident,
            "g1d": st["g1"], "g2d": st["g2"], "xdd": st["xd"],
            "segd": st["seg"], "ftgd": st["ftg"], "fexpd": st["fexp"],
            "tgtlocd": st["tgtloc"],
        })
    return in_maps


def unshard(dims, cores, results):
    NTGT = dims["NTGT"]
    out = np.zeros((NTGT, D), np.float32)
    for c in range(NCORES):
        st = cores[c]
        o = results[c]["out"]
        tw = st["twin"]
        for w2 in range(dims["W2"]):
            sel = tw[w2] >= 0
            out[tw[w2][sel]] = o[w2 * WS2 : w2 * WS2 + WS2][sel]
    return out


def kernel(**inputs):
    from concourse.bass_utils import run_bass_kernel_spmd

    h_v = np.asarray(inputs["h_v"], np.float32)
    h_p = np.asarray(inputs["h_p"], np.float32)
    h_t = np.asarray(inputs["h_t"], np.float32)
    pi_w = np.asarray(inputs["pi_w"], np.float32)
    q_w = np.asarray(inputs["q_w"], np.float32)
    r_w = np.asarray(inputs["r_w"], np.float32)
    int_src = np.asarray(inputs["int_src"]).astype(np.int64)
    int_dst = np.asarray(inputs["int_dst"]).astype(np.int64)
    agg_src = np.asarray(inputs["agg_src"]).astype(np.int64)
    agg_dst = np.asarray(inputs["agg_dst"]).astype(np.int64)
    assert np.array_equal(agg_src, np.arange(agg_src.shape[0])), \
        "kernel assumes agg_src == arange (per problem spec fill)"

    dims, cores = preprocess(h_v, h_p, h_t, int_src, int_dst, agg_dst)
    nc = build_program(dims)
    global _LAST_NC
    _LAST_NC = nc
    in_maps = make_in_maps(dims, cores, h_v, pi_w, q_w, r_w)
    res = run_bass_kernel_spmd(nc, in_maps, core_ids=list(range(NCORES)))
    return unshard(dims, cores, res.results)

